# revision 17
# baseline (speedup 1.0000x reference)
"""Trainium2 Bass kernel: MLP embedding + cosine-similarity affinity + sigmoid.

Computes, for full inputs feat1/feat2 [8192, 2048] and a shared 3-layer MLP
(2048->256->128->64):
    e1 = MLP(feat1); e2 = MLP(feat2)
    n1 = e1 / max(||e1||, 1e-8); n2 = e2 / max(||e2||, 1e-8)
    out = sigmoid(5 * n1 @ n2.T)            # [8192, 8192] f32

Sharding: rows of feat1 AND feat2 are split across 8 NeuronCores (1024 rows
each). Each core runs the MLP for its feat1 and feat2 shards in transposed
layout (features on partitions), all-gathers the normalized n2^T shards
(bf16, 128KB/core, split in two for earlier overlap), then computes its
[1024, 8192] tile of the output.

Numerics: feature matrices are transposed host-side and matmul inputs
pre-rounded to float32r (fp32 exponent, 11-bit mantissa; TensorEngine runs
f32r at 1 cycle/column vs fp32's 4). The unit-norm embeddings feeding the
affinity matmul are bf16 (error ~2e-4 on the sigmoid output).
"""

import numpy as np

import concourse.bass as bass
import concourse.mybir as mybir
import concourse.tile as tile
from concourse import bacc
from concourse.bass_utils import run_bass_kernel_spmd

F32 = mybir.dt.float32
F32R = mybir.dt.float32r
BF16 = mybir.dt.bfloat16
AF = mybir.ActivationFunctionType

CORES = 8
N1, N2, NZ = 8192, 8192, 2048
SH1 = N1 // CORES   # 1024 rows of feat1 per core
SH2 = N2 // CORES   # 1024 rows of feat2 per core
H1, H2, H3 = 256, 128, 64
ROWB = 512          # row block per MLP pass
NBLK1 = SH1 // ROWB
NBLK2 = SH2 // ROWB
KT = NZ // 128      # 16 contraction chunks for layer 1
KC = 4              # xt DMA chunks per block (KT/KC k-slices each)
KPC = KT // KC
EPS = 1e-8


def _round_f32r(a):
    """Round-to-nearest-even fp32 -> fp32r (drop low 12 mantissa bits)."""
    u = np.ascontiguousarray(a, dtype=np.float32).view(np.uint32).astype(np.uint64)
    u = (u + 0x7FF + ((u >> 12) & 1)) & 0xFFFFF000
    return u.astype(np.uint32).view(np.float32)


def _mlp_block(nc, pools, xT_cols_ap, nT_dest, w1_chunk_dmas=None):
    """MLP for one 512-row block (given as transposed DRAM cols) -> n^T slice.

    xT_cols_ap: DRAM AP [2048, 512] (featT columns for this block), f32r
    nT_dest:    SBUF AP [64, 512] (bf16) to receive normalized e^T
    w1_chunk_dmas: for the first block, W1-chunk DMA emitters to interleave
                   with the xt chunk loads (keeps the sync-ring FIFO warm).
    """
    (consts, xt_pool, hsb, esb, nrm_pool, ph1, psmall) = pools
    w1_sb = consts["w1"]; w2_sb = consts["w2"]; w3_sb = consts["w3"]
    b1_sb = consts["b1"]; b2_sb = consts["b2"]; b3_sb = consts["b3"]
    ones64 = consts["ones64"]; ones1 = consts["ones1"]

    # 4x 1MB DMA chunks so the first L1 matmul starts after ~1MB, not 4MB.
    xts = []
    for c in range(KC):
        if w1_chunk_dmas:
            w1_chunk_dmas[c]()
        xc = xt_pool.tile([128, KPC, ROWB], F32R, tag="xt", name=f"xt_{c}")
        src = xT_cols_ap[c * KPC * 128:(c + 1) * KPC * 128, :]
        nc.sync.dma_start(out=xc, in_=src.rearrange("(k p) c -> p k c", p=128))
        xts.append(xc)

    # Layer 1: h1^T [256, 512] = relu(W1^T x^T + b1), two 128-partition tiles.
    h1_tiles = []
    for m in range(2):
        pm = ph1.tile([128, ROWB], F32, tag="h1", name=f"ph1_{m}")
        for k in range(KT):
            nc.tensor.matmul(
                pm,
                w1_sb[:, k, 128 * m:128 * (m + 1)],
                xts[k // KPC][:, k % KPC, :],
                start=(k == 0),
                stop=(k == KT - 1),
            )
        h1m = hsb.tile([128, ROWB], F32R, tag="h1sb", name=f"h1_{m}")
        nc.scalar.activation(h1m, pm, AF.Relu, bias=b1_sb[:, m:m + 1])
        h1_tiles.append(h1m)

    # Layer 2: h2^T [128, 512] = relu(W2^T h1^T + b2)
    p2 = psmall.tile([128, ROWB], F32, tag="s", name="p2")
    for k2 in range(2):
        nc.tensor.matmul(
            p2,
            w2_sb[:, k2, :],
            h1_tiles[k2],
            start=(k2 == 0),
            stop=(k2 == 1),
        )
    h2t = hsb.tile([128, ROWB], F32R, tag="h2sb", name="h2t")
    nc.scalar.activation(h2t, p2, AF.Relu, bias=b2_sb)

    # Layer 3: e^T [64, 512] = W3^T h2^T + b3
    pe_ = psmall.tile([H3, ROWB], F32, tag="s", name="pe_")
    nc.tensor.matmul(pe_, w3_sb, h2t, start=True, stop=True)
    eT = esb.tile([H3, ROWB], F32, tag="e", name="eT")
    nc.vector.tensor_scalar_add(eT, pe_, b3_sb)

    # Row norms: sumsq via ones-matmul (partition reduce), sqrt, clamp, recip.
    sq = esb.tile([H3, ROWB], F32, tag="sq", name="sq")
    nc.vector.tensor_mul(sq, eT, eT)
    pss = psmall.tile([1, ROWB], F32, tag="s", name="pss")
    nc.tensor.matmul(pss, ones64, sq, start=True, stop=True)
    nrm = nrm_pool.tile([1, ROWB], F32, tag="nrm", name="nrm")
    nc.scalar.activation(nrm, pss, AF.Sqrt)
    nc.vector.tensor_scalar_max(nrm, nrm, EPS)
    inv = nrm_pool.tile([1, ROWB], F32, tag="inv", name="inv")
    nc.vector.reciprocal(inv, nrm)

    # Broadcast inv across 64 partitions via outer product, then scale e^T.
    pbc = psmall.tile([H3, ROWB], F32, tag="s", name="pbc")
    nc.tensor.matmul(pbc, ones1, inv, start=True, stop=True)
    nc.vector.tensor_mul(nT_dest, eT, pbc)


def _build():
    nc = bacc.Bacc(
        "TRN2",
        target_bir_lowering=False,
        debug=False,
        num_devices=CORES,
    )
    f1T = nc.dram_tensor("f1T", [NZ, SH1], F32R, kind="ExternalInput").ap()
    f2T = nc.dram_tensor("f2T", [NZ, SH2], F32R, kind="ExternalInput").ap()
    w1 = nc.dram_tensor("W1", [NZ, H1], F32R, kind="ExternalInput").ap()
    b1 = nc.dram_tensor("b1", [H1], F32, kind="ExternalInput").ap()
    w2 = nc.dram_tensor("W2", [H1, H2], F32R, kind="ExternalInput").ap()
    b2 = nc.dram_tensor("b2", [H2], F32, kind="ExternalInput").ap()
    w3 = nc.dram_tensor("W3", [H2, H3], F32R, kind="ExternalInput").ap()
    b3 = nc.dram_tensor("b3", [H3], F32, kind="ExternalInput").ap()
    out = nc.dram_tensor("out", [SH1, N2], F32, kind="ExternalOutput").ap()

    with tile.TileContext(nc) as tc:
        with (
            tc.tile_pool(name="consts", bufs=1) as cpool,
            tc.tile_pool(name="xt", bufs=6) as xt_pool,
            tc.tile_pool(name="hsb", bufs=3) as hsb,
            tc.tile_pool(name="esb", bufs=2) as esb,
            tc.tile_pool(name="nrm", bufs=2) as nrm_pool,
            tc.tile_pool(name="nloc", bufs=1) as nloc,
            tc.tile_pool(name="n2f", bufs=1) as n2f,
            tc.tile_pool(name="outp", bufs=3) as outp,
            tc.tile_pool(name="dram", bufs=1, space="DRAM") as dram,
        ):
            # ---- constants; W1 split into 4 chunk-DMAs interleaved with the
            # first block's xt loads so the first L1 matmul starts ASAP ----
            w1_sb = cpool.tile([128, KT, H1], F32R, tag="w1", name="w1_sb")

            def _w1_dma(c):
                def emit():
                    nc.sync.dma_start(
                        out=w1_sb[:, c * KPC:(c + 1) * KPC, :],
                        in_=w1[c * KPC * 128:(c + 1) * KPC * 128, :].rearrange(
                            "(k p) c -> p k c", p=128))
                return emit

            w1_dmas = [_w1_dma(c) for c in range(KC)]
            w2_sb = cpool.tile([128, 2, H2], F32R, tag="w2", name="w2_sb")
            w3_sb = cpool.tile([H2, H3], F32R, tag="w3", name="w3_sb")
            b1_sb = cpool.tile([128, 2], F32, tag="b1", name="b1_sb")
            b2_sb = cpool.tile([H2, 1], F32, tag="b2", name="b2_sb")
            b3_sb = cpool.tile([H3, 1], F32, tag="b3", name="b3_sb")
            # small consts on the scalar ring so they don't delay xt chunks
            nc.scalar.dma_start(
                out=w2_sb, in_=w2.rearrange("(k p) c -> p k c", p=128))
            nc.scalar.dma_start(out=w3_sb, in_=w3)
            nc.scalar.dma_start(out=b1_sb, in_=b1.rearrange("(m p) -> p m", p=128))
            nc.scalar.dma_start(out=b2_sb, in_=b2.rearrange("(p c) -> p c", c=1))
            nc.scalar.dma_start(out=b3_sb, in_=b3.rearrange("(p c) -> p c", c=1))
            ones64 = cpool.tile([H3, 1], F32, tag="ones64", name="ones64")
            nc.vector.memset(ones64, 1.0)
            ones1 = cpool.tile([1, H3], F32, tag="ones1", name="ones1")
            nc.vector.memset(ones1, 1.0)
            consts = {
                "w1": w1_sb, "w2": w2_sb, "w3": w3_sb,
                "b1": b1_sb, "b2": b2_sb, "b3": b3_sb,
                "ones64": ones64, "ones1": ones1,
            }

            n1T = nloc.tile([H3, SH1], BF16, tag="n1", name="n1T")
            n2loc = nloc.tile([H3, SH2], BF16, tag="n2", name="n2loc")
            # n2full slot s = global feat2 rows [512s, 512(s+1)): the host
            # hands core j rows {512j..} and {4096+512j..}, so AG{rb} returns
            # the contiguous half [4096rb, 4096(rb+1)) stacked by rank.
            n2full = n2f.tile([H3, 2 * CORES, ROWB], BF16, tag="n2f",
                              name="n2full")

            cc_ins, cc_outs = [], []
            for rb in range(NBLK2):
                ci = dram.tile([H3, ROWB], BF16, tag=f"ccin{rb}",
                               name=f"cc_in{rb}")
                co = dram.tile([CORES * H3, ROWB], BF16, tag=f"ccout{rb}",
                               name=f"cc_out{rb}")
                cc_ins.append(ci); cc_outs.append(co)

            # Tiny warmup AllGather issued first: pays the communicator
            # entry-barrier + first-collective warmup concurrently with the
            # MLP so the real gathers run at warmed speed.
            warm_in = dram.tile([H3, 1], F32, tag="win", name="warm_in")
            warm_out = dram.tile([CORES * H3, 1], F32, tag="wout",
                                 name="warm_out")
            nc.scalar.dma_start(
                out=warm_in, in_=b3.rearrange("(p c) -> p c", c=1))
            nc.gpsimd.collective_compute(
                "AllGather",
                mybir.AluOpType.bypass,
                replica_groups=[list(range(CORES))],
                ins=[warm_in.opt()],
                outs=[warm_out.opt()],
            )

            # ---- phases A+B (MLP) and the split all-gather ----
            with (
                tc.tile_pool(name="ph1", bufs=2, space="PSUM") as ph1,
                tc.tile_pool(name="psmall", bufs=2, space="PSUM") as psmall,
            ):
                pools = (consts, xt_pool, hsb, esb, nrm_pool, ph1, psmall)

                # phase A: e2 for this core's feat2 shard; gather each
                # 512-row half as soon as it is ready (cc DMAs ride the
                # scalar ring so the sync ring keeps streaming xt tiles).
                for rb in range(NBLK2):
                    _mlp_block(
                        nc, pools,
                        f2T[:, rb * ROWB:(rb + 1) * ROWB],
                        n2loc[:, rb * ROWB:(rb + 1) * ROWB],
                        w1_chunk_dmas=w1_dmas if rb == 0 else None,
                    )
                    nc.scalar.dma_start(
                        out=cc_ins[rb],
                        in_=n2loc[:, rb * ROWB:(rb + 1) * ROWB])
                    nc.gpsimd.collective_compute(
                        "AllGather",
                        mybir.AluOpType.bypass,
                        replica_groups=[list(range(CORES))],
                        ins=[cc_ins[rb].opt()],
                        outs=[cc_outs[rb].opt()],
                    )

                # phase B: e1 for this core's feat1 shard (overlaps gathers)
                for rb in range(NBLK1):
                    _mlp_block(
                        nc, pools,
                        f1T[:, rb * ROWB:(rb + 1) * ROWB],
                        n1T[:, rb * ROWB:(rb + 1) * ROWB],
                    )

            # load gathered n2^T halves: cc_out{rb}[64j + p, c] holds rank j,
            # feat2 row 4096rb + 512j + c, feature p.
            for rb in range(NBLK2):
                nc.scalar.dma_start(
                    out=n2full[:, rb * CORES:(rb + 1) * CORES, :],
                    in_=cc_outs[rb].rearrange("(j p) c -> p j c", p=H3),
                )

            # ---- phase C: affinity + sigmoid + store ----
            NCOL = N2 // ROWB            # 16 column blocks of 512
            PAW = 4                      # 512-blocks per PSUM tile (4 banks)
            GCOL = 8                     # 512-blocks per output tile
            with tc.tile_pool(name="paff", bufs=2, space="PSUM") as paff:
                # g outer: g=0 (left half) depends only on AG0; all of it is
                # emitted first so AG1 hides entirely under it.
                for g in range(NCOL // GCOL):
                    for m in range(SH1 // 128):  # 8 row tiles of 128
                        lhsT = n1T[:, 128 * m:128 * (m + 1)]
                        ot = outp.tile([128, GCOL * ROWB], F32, tag="o",
                                       name=f"ot_{m}_{g}")
                        for h in range(GCOL // PAW):
                            pa = paff.tile([128, PAW * ROWB], F32, tag="aff",
                                           name=f"pa_{m}_{g}_{h}")
                            for j in range(PAW):
                                n = g * GCOL + h * PAW + j
                                nc.tensor.matmul(
                                    pa[:, j * ROWB:(j + 1) * ROWB],
                                    lhsT,
                                    n2full[:, n, :],
                                    start=True,
                                    stop=True,
                                )
                            nc.scalar.activation(
                                ot[:, h * PAW * ROWB:(h + 1) * PAW * ROWB],
                                pa, AF.Sigmoid, scale=5.0)
                        eng = nc.sync if (2 * g + m) % 2 == 0 else nc.scalar
                        eng.dma_start(
                            out=out[128 * m:128 * (m + 1),
                                    g * GCOL * ROWB:(g + 1) * GCOL * ROWB],
                            in_=ot,
                        )

    nc.compile()
    return nc


_NC_CACHE = []


def _get_nc():
    if not _NC_CACHE:
        _NC_CACHE.append(_build())
    return _NC_CACHE[0]


def _run(inputs, trace=False, **kw):
    nc = _get_nc()

    def f32c(a):
        return np.ascontiguousarray(np.asarray(a, dtype=np.float32))

    full = {k: f32c(v) for k, v in inputs.items()}
    f1T = _round_f32r(full["feat1"].T)   # [2048, 8192]
    f2T = _round_f32r(full["feat2"].T)
    w1r, w2r, w3r = (_round_f32r(full[k]) for k in ("W1", "W2", "W3"))
    in_maps = []
    for i in range(CORES):
        # feat2 shard = global half-blocks i and 8+i, so each AllGather
        # returns a contiguous 4096-column half of the affinity output.
        f2T_i = np.concatenate(
            [f2T[:, 512 * i:512 * (i + 1)],
             f2T[:, 4096 + 512 * i:4096 + 512 * (i + 1)]], axis=1)
        in_maps.append({
            "f1T": np.ascontiguousarray(f1T[:, i * SH1:(i + 1) * SH1]),
            "f2T": np.ascontiguousarray(f2T_i),
            "W1": w1r, "b1": full["b1"],
            "W2": w2r, "b2": full["b2"],
            "W3": w3r, "b3": full["b3"],
        })
    res = run_bass_kernel_spmd(
        nc, in_maps, core_ids=list(range(CORES)), trace=trace, **kw)
    out = np.concatenate(
        [np.asarray(res.results[i]["out"]) for i in range(CORES)], axis=0)
    return out, res


def kernel(**inputs):
    out, _ = _run(inputs, trace=False)
    return out


# revision 25
# speedup vs baseline: 1.2630x; 1.2630x over previous
"""Trainium2 Bass kernel: MLP embedding + cosine-similarity affinity + sigmoid.

Computes, for full inputs feat1/feat2 [8192, 2048] and a shared 3-layer MLP
(2048->256->128->64):
    e1 = MLP(feat1); e2 = MLP(feat2)
    n1 = e1 / max(||e1||, 1e-8); n2 = e2 / max(||e2||, 1e-8)
    out = sigmoid(5 * n1 @ n2.T)            # [8192, 8192] f32

Sharding: rows of feat1 AND feat2 are split across 8 NeuronCores (1024 rows
each). Each core runs the MLP for its feat1 and feat2 shards in transposed
layout (features on partitions), all-gathers the normalized n2^T shards
(bf16, 128KB/core, split in two for earlier overlap), then computes its
[1024, 8192] tile of the output.

Numerics: feature matrices are transposed host-side and matmul inputs
pre-rounded to float32r (fp32 exponent, 11-bit mantissa; TensorEngine runs
f32r at 1 cycle/column vs fp32's 4). The unit-norm embeddings feeding the
affinity matmul are bf16 (error ~2e-4 on the sigmoid output).
"""

import numpy as np

import concourse.bass as bass
import concourse.mybir as mybir
import concourse.tile as tile
from concourse import bacc
from concourse.bass_utils import run_bass_kernel_spmd

F32 = mybir.dt.float32
F32R = mybir.dt.float32r
BF16 = mybir.dt.bfloat16
AF = mybir.ActivationFunctionType

CORES = 8
N1, N2, NZ = 8192, 8192, 2048
SH1 = N1 // CORES   # 1024 rows of feat1 per core
SH2 = N2 // CORES   # 1024 rows of feat2 per core
H1, H2, H3 = 256, 128, 64
ROWB = 512          # row block per MLP pass
NBLK1 = SH1 // ROWB
NBLK2 = SH2 // ROWB
KT = NZ // 128      # 16 contraction chunks for layer 1
KC = 4              # xt DMA chunks per block (KT/KC k-slices each)
KPC = KT // KC
EPS = 1e-8
WARMUP_AG = True


def _round_f32r(a):
    """Round-to-nearest-even fp32 -> fp32r (drop low 12 mantissa bits)."""
    u = np.ascontiguousarray(a, dtype=np.float32).view(np.uint32).astype(np.uint64)
    u = (u + 0x7FF + ((u >> 12) & 1)) & 0xFFFFF000
    return u.astype(np.uint32).view(np.float32)


def _mlp_block(nc, pools, xT_cols_ap, nT_dest, w1_chunk_dmas=None):
    """MLP for one 512-row block (given as transposed DRAM cols) -> n^T slice.

    xT_cols_ap: DRAM AP [2048, 512] (featT columns for this block), f32r
    nT_dest:    SBUF AP [64, 512] (bf16) to receive normalized e^T
    w1_chunk_dmas: for the first block, W1-chunk DMA emitters to interleave
                   with the xt chunk loads (keeps the sync-ring FIFO warm).
    """
    (consts, xt_pool, hsb, esb, nrm_pool, ph1, psmall) = pools
    w1_sb = consts["w1"]; w2_sb = consts["w2"]; w3_sb = consts["w3"]
    b1_sb = consts["b1"]; b2_sb = consts["b2"]; b3_sb = consts["b3"]
    ones64 = consts["ones64"]; ones1 = consts["ones1"]

    # 4x 1MB DMA chunks so the first L1 matmul starts after ~1MB, not 4MB;
    # chunks alternate across the two HWDGE rings to hide per-DMA gaps.
    xts = []
    for c in range(KC):
        if w1_chunk_dmas:
            w1_chunk_dmas[c]()
        xc = xt_pool.tile([128, KPC, ROWB], F32R, tag="xt", name=f"xt_{c}")
        src = xT_cols_ap[c * KPC * 128:(c + 1) * KPC * 128, :]
        eng = nc.sync if c % 2 == 0 else nc.scalar
        eng.dma_start(out=xc, in_=src.rearrange("(k p) c -> p k c", p=128))
        xts.append(xc)

    # Layer 1: h1^T [256, 512] = relu(W1^T x^T + b1), two 128-partition tiles.
    h1_tiles = []
    for m in range(2):
        pm = ph1.tile([128, ROWB], F32, tag="h1", name=f"ph1_{m}")
        for k in range(KT):
            nc.tensor.matmul(
                pm,
                w1_sb[:, k, 128 * m:128 * (m + 1)],
                xts[k // KPC][:, k % KPC, :],
                start=(k == 0),
                stop=(k == KT - 1),
            )
        h1m = hsb.tile([128, ROWB], F32R, tag="h1sb", name=f"h1_{m}")
        nc.scalar.activation(h1m, pm, AF.Relu, bias=b1_sb[:, m:m + 1])
        h1_tiles.append(h1m)

    # Layer 2: h2^T [128, 512] = relu(W2^T h1^T + b2)
    p2 = psmall.tile([128, ROWB], F32, tag="s", name="p2")
    for k2 in range(2):
        nc.tensor.matmul(
            p2,
            w2_sb[:, k2, :],
            h1_tiles[k2],
            start=(k2 == 0),
            stop=(k2 == 1),
        )
    h2t = hsb.tile([128, ROWB], F32R, tag="h2sb", name="h2t")
    nc.scalar.activation(h2t, p2, AF.Relu, bias=b2_sb)

    # Layer 3: e^T [64, 512] = W3^T h2^T + b3
    pe_ = psmall.tile([H3, ROWB], F32, tag="s", name="pe_")
    nc.tensor.matmul(pe_, w3_sb, h2t, start=True, stop=True)
    eT = esb.tile([H3, ROWB], F32, tag="e", name="eT")
    nc.vector.tensor_scalar_add(eT, pe_, b3_sb)

    # Row norms: sumsq via ones-matmul (partition reduce), sqrt, clamp, recip.
    # sq = (pe_ + b3)^2 on ACT runs concurrently with the DVE eT add above.
    sq = esb.tile([H3, ROWB], F32, tag="sq", name="sq")
    nc.scalar.activation(sq, pe_, AF.Square, bias=b3_sb)
    pss = psmall.tile([1, ROWB], F32, tag="s", name="pss")
    nc.tensor.matmul(pss, ones64, sq, start=True, stop=True)
    nrm = nrm_pool.tile([1, ROWB], F32, tag="nrm", name="nrm")
    nc.scalar.activation(nrm, pss, AF.Sqrt)
    nc.vector.tensor_scalar_max(nrm, nrm, EPS)
    inv = nrm_pool.tile([1, ROWB], F32, tag="inv", name="inv")
    nc.vector.reciprocal_approx_fast(inv, nrm)

    # Broadcast inv across 64 partitions via outer product, then scale e^T.
    pbc = psmall.tile([H3, ROWB], F32, tag="s", name="pbc")
    nc.tensor.matmul(pbc, ones1, inv, start=True, stop=True)
    nc.vector.tensor_mul(nT_dest, eT, pbc)


def _build():
    nc = bacc.Bacc(
        "TRN2",
        target_bir_lowering=False,
        debug=False,
        num_devices=CORES,
    )
    f1T = nc.dram_tensor("f1T", [NZ, SH1], F32R, kind="ExternalInput").ap()
    f2T = nc.dram_tensor("f2T", [NZ, SH2], F32R, kind="ExternalInput").ap()
    w1 = nc.dram_tensor("W1", [NZ, H1], F32R, kind="ExternalInput").ap()
    b1 = nc.dram_tensor("b1", [H1], F32, kind="ExternalInput").ap()
    w2 = nc.dram_tensor("W2", [H1, H2], F32R, kind="ExternalInput").ap()
    b2 = nc.dram_tensor("b2", [H2], F32, kind="ExternalInput").ap()
    w3 = nc.dram_tensor("W3", [H2, H3], F32R, kind="ExternalInput").ap()
    b3 = nc.dram_tensor("b3", [H3], F32, kind="ExternalInput").ap()
    out = nc.dram_tensor("out", [SH1, N2], F32, kind="ExternalOutput").ap()

    with tile.TileContext(nc) as tc:
        with (
            tc.tile_pool(name="consts", bufs=1) as cpool,
            tc.tile_pool(name="xt", bufs=6) as xt_pool,
            tc.tile_pool(name="hsb", bufs=3) as hsb,
            tc.tile_pool(name="esb", bufs=2) as esb,
            tc.tile_pool(name="nrm", bufs=2) as nrm_pool,
            tc.tile_pool(name="nloc", bufs=1) as nloc,
            tc.tile_pool(name="n2f", bufs=1) as n2f,
            tc.tile_pool(name="outp", bufs=3) as outp,
            tc.tile_pool(name="dram", bufs=1, space="DRAM") as dram,
        ):
            # ---- constants; W1 split into 4 chunk-DMAs interleaved with the
            # first block's xt loads so the first L1 matmul starts ASAP ----
            w1_sb = cpool.tile([128, KT, H1], F32R, tag="w1", name="w1_sb")

            def _w1_dma(c):
                def emit():
                    eng = nc.scalar if c % 2 == 0 else nc.sync
                    eng.dma_start(
                        out=w1_sb[:, c * KPC:(c + 1) * KPC, :],
                        in_=w1[c * KPC * 128:(c + 1) * KPC * 128, :].rearrange(
                            "(k p) c -> p k c", p=128))
                return emit

            w1_dmas = [_w1_dma(c) for c in range(KC)]
            _orig_last = w1_dmas[KC - 1]

            def _last_plus_consts():
                _orig_last()
                _emit_small_consts()
            w1_dmas[KC - 1] = _last_plus_consts
            w2_sb = cpool.tile([128, 2, H2], F32R, tag="w2", name="w2_sb")
            w3_sb = cpool.tile([H2, H3], F32R, tag="w3", name="w3_sb")
            b1_sb = cpool.tile([128, 2], F32, tag="b1", name="b1_sb")
            b2_sb = cpool.tile([H2, 1], F32, tag="b2", name="b2_sb")
            b3_sb = cpool.tile([H3, 1], F32, tag="b3", name="b3_sb")
            # small consts are emitted after the first block's big chunks
            # (they are not needed until the first ACT relu ~35us in)
            def _emit_small_consts():
                nc.scalar.dma_start(
                    out=w2_sb, in_=w2.rearrange("(k p) c -> p k c", p=128))
                nc.scalar.dma_start(out=w3_sb, in_=w3)
                nc.scalar.dma_start(
                    out=b1_sb, in_=b1.rearrange("(m p) -> p m", p=128))
                nc.scalar.dma_start(
                    out=b2_sb, in_=b2.rearrange("(p c) -> p c", c=1))
                nc.scalar.dma_start(
                    out=b3_sb, in_=b3.rearrange("(p c) -> p c", c=1))
            ones64 = cpool.tile([H3, 1], F32, tag="ones64", name="ones64")
            nc.vector.memset(ones64, 1.0)
            ones1 = cpool.tile([1, H3], F32, tag="ones1", name="ones1")
            nc.vector.memset(ones1, 1.0)
            consts = {
                "w1": w1_sb, "w2": w2_sb, "w3": w3_sb,
                "b1": b1_sb, "b2": b2_sb, "b3": b3_sb,
                "ones64": ones64, "ones1": ones1,
            }

            n1T = nloc.tile([H3, SH1], BF16, tag="n1", name="n1T")
            n2loc = nloc.tile([H3, SH2], BF16, tag="n2", name="n2loc")
            # n2full slot s = global feat2 rows [512s, 512(s+1)): the host
            # hands core j rows {512j..} and {4096+512j..}, so AG{rb} returns
            # the contiguous half [4096rb, 4096(rb+1)) stacked by rank.
            n2full = n2f.tile([H3, 2 * CORES, ROWB], BF16, tag="n2f",
                              name="n2full")

            cc_ins, cc_outs = [], []
            for rb in range(NBLK2):
                ci = dram.tile([H3, ROWB], BF16, tag=f"ccin{rb}",
                               name=f"cc_in{rb}")
                co = dram.tile([CORES * H3, ROWB], BF16, tag=f"ccout{rb}",
                               name=f"cc_out{rb}")
                cc_ins.append(ci); cc_outs.append(co)

            # Tiny warmup AllGather issued first: pays the communicator
            # entry-barrier + first-collective warmup concurrently with the
            # MLP so the real gathers run at warmed speed.
            warm_in = dram.tile([H3, 1], F32, tag="win", name="warm_in")
            warm_out = dram.tile([CORES * H3, 1], F32, tag="wout",
                                 name="warm_out")
            if WARMUP_AG:
                # contents are irrelevant: this exists only to pull the
                # communicator entry barrier + first-collective warmup to t=0
                nc.gpsimd.collective_compute(
                    "AllGather",
                    mybir.AluOpType.bypass,
                    replica_groups=[list(range(CORES))],
                    ins=[warm_in.opt()],
                    outs=[warm_out.opt()],
                )

            # ---- phases A+B (MLP) and the split all-gather ----
            with (
                tc.tile_pool(name="ph1", bufs=2, space="PSUM") as ph1,
                tc.tile_pool(name="psmall", bufs=2, space="PSUM") as psmall,
            ):
                pools = (consts, xt_pool, hsb, esb, nrm_pool, ph1, psmall)

                # phase A: e2 for this core's feat2 shard; gather each
                # 512-row half as soon as it is ready (cc bounce DMAs ride
                # gpsimd/SWDGE so no HWDGE ring ever stalls behind them).
                for rb in range(NBLK2):
                    _mlp_block(
                        nc, pools,
                        f2T[:, rb * ROWB:(rb + 1) * ROWB],
                        n2loc[:, rb * ROWB:(rb + 1) * ROWB],
                        w1_chunk_dmas=w1_dmas if rb == 0 else None,
                    )
                    nc.gpsimd.dma_start(
                        out=cc_ins[rb],
                        in_=n2loc[:, rb * ROWB:(rb + 1) * ROWB])
                    nc.gpsimd.collective_compute(
                        "AllGather",
                        mybir.AluOpType.bypass,
                        replica_groups=[list(range(CORES))],
                        ins=[cc_ins[rb].opt()],
                        outs=[cc_outs[rb].opt()],
                    )

                # phase B: e1 for this core's feat1 shard (overlaps gathers)
                for rb in range(NBLK1):
                    _mlp_block(
                        nc, pools,
                        f1T[:, rb * ROWB:(rb + 1) * ROWB],
                        n1T[:, rb * ROWB:(rb + 1) * ROWB],
                    )

            # load gathered n2^T halves: cc_out{rb}[64j + p, c] holds rank j,
            # feat2 row 4096rb + 512j + c, feature p.
            for rb in range(NBLK2):
                nc.scalar.dma_start(
                    out=n2full[:, rb * CORES:(rb + 1) * CORES, :],
                    in_=cc_outs[rb].rearrange("(j p) c -> p j c", p=H3),
                )

            # ---- phase C: affinity + sigmoid + store ----
            NCOL = N2 // ROWB            # 16 column blocks of 512
            PAW = 2                      # 512-blocks per PSUM tile (2 banks)
            GCOL = 8                     # 512-blocks per output tile
            with tc.tile_pool(name="paff", bufs=4, space="PSUM") as paff:
                # g outer: g=0 (left half) depends only on AG0; all of it is
                # emitted first so AG1 hides entirely under it.
                for g in range(NCOL // GCOL):
                    for m in range(SH1 // 128):  # 8 row tiles of 128
                        lhsT = n1T[:, 128 * m:128 * (m + 1)]
                        ot = outp.tile([128, GCOL * ROWB], F32, tag="o",
                                       name=f"ot_{m}_{g}")
                        for h in range(GCOL // PAW):
                            pa = paff.tile([128, PAW * ROWB], F32, tag="aff",
                                           name=f"pa_{m}_{g}_{h}")
                            for j in range(PAW):
                                n = g * GCOL + h * PAW + j
                                nc.tensor.matmul(
                                    pa[:, j * ROWB:(j + 1) * ROWB],
                                    lhsT,
                                    n2full[:, n, :],
                                    start=True,
                                    stop=True,
                                )
                            nc.scalar.activation(
                                ot[:, h * PAW * ROWB:(h + 1) * PAW * ROWB],
                                pa, AF.Sigmoid, scale=5.0)
                        eng = nc.sync if (2 * g + m) % 2 == 0 else nc.scalar
                        eng.dma_start(
                            out=out[128 * m:128 * (m + 1),
                                    g * GCOL * ROWB:(g + 1) * GCOL * ROWB],
                            in_=ot,
                        )

    nc.compile()
    return nc


_NC_CACHE = []


def _get_nc():
    if not _NC_CACHE:
        _NC_CACHE.append(_build())
    return _NC_CACHE[0]


def _run(inputs, trace=False, **kw):
    nc = _get_nc()

    def f32c(a):
        return np.ascontiguousarray(np.asarray(a, dtype=np.float32))

    full = {k: f32c(v) for k, v in inputs.items()}
    f1T = _round_f32r(full["feat1"].T)   # [2048, 8192]
    f2T = _round_f32r(full["feat2"].T)
    w1r, w2r, w3r = (_round_f32r(full[k]) for k in ("W1", "W2", "W3"))
    in_maps = []
    for i in range(CORES):
        # feat2 shard = global half-blocks i and 8+i, so each AllGather
        # returns a contiguous 4096-column half of the affinity output.
        f2T_i = np.concatenate(
            [f2T[:, 512 * i:512 * (i + 1)],
             f2T[:, 4096 + 512 * i:4096 + 512 * (i + 1)]], axis=1)
        in_maps.append({
            "f1T": np.ascontiguousarray(f1T[:, i * SH1:(i + 1) * SH1]),
            "f2T": np.ascontiguousarray(f2T_i),
            "W1": w1r, "b1": full["b1"],
            "W2": w2r, "b2": full["b2"],
            "W3": w3r, "b3": full["b3"],
        })
    last_err = None
    for attempt in range(3):
        try:
            res = run_bass_kernel_spmd(
                nc, in_maps, core_ids=list(range(CORES)), trace=trace, **kw)
            out = np.concatenate(
                [np.asarray(res.results[i]["out"]) for i in range(CORES)],
                axis=0)
            return out, res
        except Exception as e:  # transient NRT/device hiccups: retry
            last_err = e
            import time
            time.sleep(2.0)
    raise last_err


def kernel(**inputs):
    out, _ = _run(inputs, trace=False)
    return out
